# revision 10
# baseline (speedup 1.0000x reference)
"""Trainium2 Bass kernel for nn_Attention_48876727828718.

RBF-kernel causal attention with per-head full-rank projections:
  xn = LayerNorm(x) * ln_w
  Q/K/V = xn @ W_{q,k,v}[h]          (per head, [S,E]@[E,E])
  scores = exp(-gamma_h * ||q_i - k_j||^2 / sqrt(E)) * causal
  out = (scores @ V  concat heads) @ W_o.T

Sharding: B(2) x H(8) = 16 (b,h) pairs over 8 cores; core c handles
batch b = c//4 and heads {2*(c%4), 2*(c%4)+1}.  Host sums the 4 partial
outputs per batch (W_o is folded into V on device via Wvo = W_v @ W_o_blk^T).

Device algorithm per (b, h) — flash-style, scores never touch HBM:
  - LN in rows layout [128, 16*64], PE-transpose to xnT [64, 2048]
  - QT/KT via matmul(lhsT=W[h], rhs=xnT) into augmented [128, S] operands
    (rows 0/32 hold the -q2/2 / ones stat rows) so one K=128 matmul yields
    A[j,q] = Q_q.K_j - q2[q]/2 - k2[j]/2  (= -sqdist/2)
  - T_exp[j,q] = exp(2*gamma/sqrt(E) * A) via ACT (scale = per-partition AP)
  - causal mask via gpsimd affine_select on the diagonal slice
  - OUT[e,q] += VW_j^T @ T_exp  accumulated over (h, j) in PSUM per q-super,
    where VW = xn @ (W_v @ W_o_blk^T)  -- W_o applied for free
  - two q-super passes (supers {0,1} then {2,3}) so PSUM splits into
    independent pools: T-chunks (4 banks) / OT accum (2) / transients (2)
"""

import numpy as np

B, S, E, H = 2, 2048, 64, 8
EPS = 1e-5
NCORES = 8
USE_F32R = True  # float32r matmuls: 4x faster streaming on TRN2 for N>=256

_BUILT = {}


def _build(use_f32r: bool):
    """Build + compile the single-core Bass program (same NEFF for all cores)."""
    from contextlib import ExitStack

    import concourse.bass as bass
    import concourse.mybir as mybir
    import concourse.tile as tile
    from concourse import bacc
    from concourse.masks import make_identity

    fp32 = mybir.dt.float32
    f32r = mybir.dt.float32r
    Exp = mybir.ActivationFunctionType.Exp
    Sqrt = mybir.ActivationFunctionType.Sqrt
    Square = mybir.ActivationFunctionType.Square
    Copy = mybir.ActivationFunctionType.Copy
    X = mybir.AxisListType.X
    add = mybir.AluOpType.add
    mult = mybir.AluOpType.mult
    is_ge = mybir.AluOpType.is_ge

    def mm(ap):
        return ap.bitcast(f32r) if use_f32r else ap

    rr = mm  # writers of matmul-feeding tiles must emit f32r-rounded values

    nc = bacc.Bacc("TRN2", target_bir_lowering=False, debug=False)

    x_d = nc.dram_tensor("x", [S, E], fp32, kind="ExternalInput").ap()
    wq_d = nc.dram_tensor("wq", [2, E, E], fp32, kind="ExternalInput").ap()
    wk_d = nc.dram_tensor("wk", [2, E, E], fp32, kind="ExternalInput").ap()
    wvo_d = nc.dram_tensor("wvo", [2, E, E], fp32, kind="ExternalInput").ap()
    gsc_d = nc.dram_tensor("gsc", [2, 128], fp32, kind="ExternalInput").ap()
    aug0_d = nc.dram_tensor("aug0", [2, E, S], fp32, kind="ExternalInput").ap()
    out_d = nc.dram_tensor("out", [E, S], fp32, kind="ExternalOutput").ap()

    NB = S // 128  # 16 j/row blocks
    NQ = S // 512  # 4 q-super blocks

    with ExitStack() as ctx:
        tc = ctx.enter_context(tile.TileContext(nc))
        const = ctx.enter_context(tc.tile_pool(name="const", bufs=1))
        sb = ctx.enter_context(tc.tile_pool(name="sb", bufs=1))
        hb = ctx.enter_context(tc.tile_pool(name="hb", bufs=1))
        texp_pool = ctx.enter_context(tc.tile_pool(name="texp", bufs=4))
        ps_T = ctx.enter_context(tc.tile_pool(name="psT", bufs=2, space="PSUM"))
        ps_ot = ctx.enter_context(tc.tile_pool(name="psot", bufs=2, space="PSUM"))
        ps_tr = ctx.enter_context(tc.tile_pool(name="pstr", bufs=2, space="PSUM"))

        # ---- constants ----
        identity = const.tile([128, 128], fp32)
        make_identity(nc, identity)
        neghalf = const.tile([64, 1], fp32)
        nc.gpsimd.memset(neghalf, -0.5)
        zero_col = const.tile([128, 1], fp32)
        nc.gpsimd.memset(zero_col, 0.0)
        eps_col = const.tile([128, 1], fp32)
        nc.gpsimd.memset(eps_col, EPS)
        nc.const_aps.aps[(fp32, 0.0)] = zero_col
        nc.const_aps.aps[(fp32, EPS)] = eps_col
        # weights: dest[e, h, f] = W[h, e, f]; DMA to staging then round
        # to f32r via DVE copy (matmul consumers require rounded producers)
        wq_st = const.tile([E, 2 * E], fp32)
        nc.sync.dma_start(
            wq_st.rearrange("e (h f) -> e h f", h=2), wq_d.transpose([1, 0, 2])
        )
        wk_st = const.tile([E, 2 * E], fp32)
        nc.sync.dma_start(
            wk_st.rearrange("e (h f) -> e h f", h=2), wk_d.transpose([1, 0, 2])
        )
        wvo_st = const.tile([E, 2 * E], fp32)
        nc.sync.dma_start(
            wvo_st.rearrange("e (h f) -> e h f", h=2), wvo_d.transpose([1, 0, 2])
        )
        wq_sb = const.tile([E, 2 * E], fp32)
        nc.vector.tensor_copy(rr(wq_sb), wq_st)
        wk_sb = const.tile([E, 2 * E], fp32)
        nc.vector.tensor_copy(rr(wk_sb), wk_st)
        wvo_sb = const.tile([E, 2 * E], fp32)
        nc.vector.tensor_copy(rr(wvo_sb), wvo_st)
        gsc_sb = const.tile([128, 2], fp32)
        nc.sync.dma_start(gsc_sb, gsc_d.transpose([1, 0]))

        # ---- LayerNorm (ln_w folded into weights host-side) ----
        x_sb = sb.tile([128, NB * E], fp32)
        nc.sync.dma_start(
            x_sb.rearrange("p (n e) -> p n e", e=E),
            x_d.rearrange("(n p) e -> p n e", p=128),
        )
        # PE warmer: keep TensorE busy through the LN phase so HAM
        # un-throttles (K=8/8) before the matmul-dense main work begins.
        warm_ps = ps_tr.tile([128, 512], fp32, name="warm", tag="tr")
        for _ in range(8):
            nc.tensor.matmul(
                warm_ps, x_sb[:, 0:128], x_sb[:, 0:512], start=True, stop=True
            )
        x3 = x_sb.rearrange("p (n e) -> p n e", e=E)
        red = sb.tile([128, NB], fp32)
        nc.vector.reduce_sum(red, x3, axis=X)
        negmu = sb.tile([128, NB], fp32)
        nc.vector.tensor_scalar_mul(negmu, red, -1.0 / E)
        xm = sb.tile([128, NB * E], fp32)
        xm3 = xm.rearrange("p (n e) -> p n e", e=E)
        nc.vector.tensor_tensor(
            xm3, x3, negmu.unsqueeze(2).broadcast_to([128, NB, E]), add
        )
        xsq = sb.tile([128, NB * E], fp32)
        nc.vector.tensor_tensor(xsq, xm, xm, mult)
        vred = sb.tile([128, NB], fp32)
        nc.vector.reduce_sum(vred, xsq.rearrange("p (n e) -> p n e", e=E), axis=X)
        std = sb.tile([128, NB], fp32)
        nc.scalar.activation(std, vred, Sqrt, bias=EPS, scale=1.0 / E)
        rstd = sb.tile([128, NB], fp32)
        nc.vector.reciprocal(rstd, std)
        xn = sb.tile([128, NB * E], fp32)
        nc.vector.tensor_tensor(
            xn.rearrange("p (n e) -> p n e", e=E),
            xm3,
            rstd.unsqueeze(2).broadcast_to([128, NB, E]),
            mult,
        )

        # ---- transpose to xnT [64, 2048] ----
        xnT = sb.tile([E, S], fp32)
        for t4 in range(4):
            tp = ps_tr.tile([64, 512], fp32, name=f"tp{t4}", tag="tr")
            for k in range(4):
                t = t4 * 4 + k
                nc.tensor.transpose(
                    tp[:, k * 128 : (k + 1) * 128],
                    xn[:, t * E : (t + 1) * E],
                    identity,
                )
            nc.vector.tensor_copy(rr(xnT[:, t4 * 512 : (t4 + 1) * 512]), tp)

        OUTsb = sb.tile([E, S], fp32)

        # ---- per-head prep: projections + stat rows + VW ----
        QT = {}
        KT = {}
        VWs = {}
        for h in range(2):
            # Engines need 32-aligned start partitions, so the augmented
            # operands are [128, S] with:
            #   QTaug: row 0 = -q2/2, row 32 = ones, rows 64:128 = Q^T
            #   KTaug: row 0 = ones,  row 32 = -k2/2, rows 64:128 = K^T
            # (rows 1:32, 33:64 are zeros; contraction over all 128 rows gives
            #  Q.K - q2/2 - k2/2 = -sqdist/2)
            QTaug = hb.tile([128, S], fp32, name=f"QTaug{h}", tag=f"qt{h}")
            KTaug = hb.tile([128, S], fp32, name=f"KTaug{h}", tag=f"kt{h}")
            QT[h], KT[h] = QTaug, KTaug
            # rows 0:64 (zeros + the ones row) come from a host constant
            # via DMA -- keeps the Pool engine off the startup critical path
            nc.sync.dma_start(rr(QTaug[0:64, :]), rr(aug0_d[0]))
            nc.sync.dma_start(rr(KTaug[0:64, :]), rr(aug0_d[1]))
            for w_sb, dst in ((wq_sb, QTaug), (wk_sb, KTaug)):
                for c4 in range(NQ):
                    pp = ps_tr.tile([64, 512], fp32, name=f"pp{h}{c4}", tag="tr")
                    nc.tensor.matmul(
                        pp,
                        mm(w_sb[:, h * E : (h + 1) * E]),
                        mm(xnT[:, c4 * 512 : (c4 + 1) * 512]),
                        start=True,
                        stop=True,
                    )
                    nc.vector.tensor_copy(
                        rr(dst[64:128, c4 * 512 : (c4 + 1) * 512]), pp
                    )
            # stat rows: -q2/2, -k2/2 via (-1/2-ones).T @ (QT*QT)
            for si, (src_t, dstrow) in enumerate(
                ((QTaug, QTaug[0:1, :]), (KTaug, KTaug[32:33, :]))
            ):
                sqt = hb.tile([E, S], fp32, name=f"sq{h}{si}", tag="sq", bufs=2)
                nc.vector.tensor_tensor(rr(sqt), src_t[64:128, :], src_t[64:128, :], mult)
                for c4 in range(NQ):
                    p2 = ps_tr.tile([1, 512], fp32, name=f"p2{h}{c4}", tag="tr")
                    nc.tensor.matmul(
                        p2,
                        mm(neghalf),
                        mm(sqt[:, c4 * 512 : (c4 + 1) * 512]),
                        start=True,
                        stop=True,
                    )
                    nc.vector.tensor_copy(
                        rr(dstrow[:, c4 * 512 : (c4 + 1) * 512]), p2
                    )

            # VW = xn @ (W_v @ W_o_blk^T), rows layout [128, 16*64]
            VW = hb.tile([128, NB * E], fp32, name=f"VW{h}", tag=f"vw{h}")
            VWs[h] = VW
            for g in range(4):
                pv = ps_tr.tile([128, 256], fp32, name=f"pv{h}{g}", tag="tr")
                for k in range(4):
                    jb = 4 * g + k
                    nc.tensor.matmul(
                        pv[:, k * E : (k + 1) * E],
                        mm(xnT[:, jb * 128 : (jb + 1) * 128]),
                        mm(wvo_sb[:, h * E : (h + 1) * E]),
                        start=True,
                        stop=True,
                    )
                nc.vector.tensor_copy(rr(VW[:, g * 256 : (g + 1) * 256]), pv)

        # ---- main loop: two q-super passes; j-blocks outer, heads
        # interleaved; both heads accumulate into the same OUT psum ----
        for sp0 in (0, 2):
            OTp = [
                ps_ot.tile([64, 512], fp32, name=f"ot{sp0}{i}", tag="ot")
                for i in range(2)
            ]
            jb_max = 8 if sp0 == 0 else NB
            for jb in range(jb_max):
                qs_first = max(sp0, jb // 4)
                qstart = 512 * qs_first
                w = 512 * (sp0 + 2) - qstart  # 512 or 1024
                has_diag = (jb // 4) >= sp0
                dead = 128 * (jb % 4) if has_diag else 0
                for h in range(2):
                    QTaug, KTaug, VW = QT[h], KT[h], VWs[h]
                    gscale = gsc_sb[:, h : h + 1]
                    tchunk = ps_T.tile([128, w], fp32, name=f"t{sp0}{h}{jb}", tag="T")
                    for s5 in range(w // 512):
                        n0 = dead if s5 == 0 else 0
                        q0 = qstart + s5 * 512
                        nc.tensor.matmul(
                            tchunk[:, s5 * 512 + n0 : (s5 + 1) * 512],
                            mm(KTaug[:, jb * 128 : (jb + 1) * 128]),
                            mm(QTaug[:, q0 + n0 : q0 + 512]),
                            start=True,
                            stop=True,
                        )
                    texp = texp_pool.tile(
                        [128, w - dead], fp32, name=f"te{sp0}{h}{jb}", tag="te"
                    )
                    nc.scalar.activation(rr(texp), tchunk[:, dead:w], Exp, scale=gscale)
                    if has_diag:
                        # causal mask on the diagonal slice: after narrowing,
                        # texp col c is global q = 128*jb + c; keep q - j >= 0
                        nc.gpsimd.affine_select(
                            out=rr(texp[:, 0 : 512 - dead]),
                            in_=rr(texp[:, 0 : 512 - dead]),
                            pattern=[[1, 512 - dead]],
                            compare_op=is_ge,
                            fill=0.0,
                            base=0,
                            channel_multiplier=-1,
                        )
                    for s5 in range(w // 512):
                        qs = qs_first + s5
                        n0 = dead if s5 == 0 else 0
                        tlo = s5 * 512 + n0 - dead
                        nc.tensor.matmul(
                            OTp[qs - sp0][:, n0:512],
                            mm(VW[:, jb * E : (jb + 1) * E]),
                            mm(texp[:, tlo : tlo + 512 - n0]),
                            start=(jb == 0 and h == 0),
                            stop=(jb == 4 * qs + 3 and h == 1),
                        )
            for i in range(2):
                qs = sp0 + i
                nc.vector.tensor_copy(OUTsb[:, qs * 512 : (qs + 1) * 512], OTp[i])

        nc.sync.dma_start(out_d, OUTsb)

    nc.compile()
    return nc


def _get_nc():
    if USE_F32R not in _BUILT:
        _BUILT[USE_F32R] = _build(USE_F32R)
    return _BUILT[USE_F32R]


def kernel(x, ln_w, W_q, W_k, W_v, W_o, gamma):
    x = np.asarray(x, np.float32)
    ln_w = np.asarray(ln_w, np.float32)
    W_q = np.asarray(W_q, np.float32)
    W_k = np.asarray(W_k, np.float32)
    W_v = np.asarray(W_v, np.float32)
    W_o = np.asarray(W_o, np.float32)
    gamma = np.asarray(gamma, np.float32).reshape(H)

    from concourse import bass_utils

    nc = _get_nc()

    # fold ln_w into projection weights; fold W_o into W_v
    lw = ln_w[None, :, None]  # [1, E, 1] scale on contraction dim e
    Wq = (W_q * lw).astype(np.float32)
    Wk = (W_k * lw).astype(np.float32)
    Wv = (W_v * lw).astype(np.float32)
    Wo_blk = W_o.reshape(E, H, E).transpose(1, 0, 2)  # [H, e_out, f]
    Wvo = np.einsum("hef,hof->heo", Wv.astype(np.float64), Wo_blk.astype(np.float64))
    Wvo = Wvo.astype(np.float32)  # [H, e, e_out]
    gs = (2.0 * gamma / np.sqrt(E)).astype(np.float32)  # exp scale per head

    aug0 = np.zeros((2, E, S), np.float32)
    aug0[0, 32, :] = 1.0  # QTaug ones row
    aug0[1, 0, :] = 1.0  # KTaug ones row

    in_maps = []
    for c in range(NCORES):
        b = c // 4
        h0 = 2 * (c % 4)
        in_maps.append(
            {
                "x": np.ascontiguousarray(x[b]),
                "wq": np.ascontiguousarray(Wq[h0 : h0 + 2]),
                "wk": np.ascontiguousarray(Wk[h0 : h0 + 2]),
                "wvo": np.ascontiguousarray(Wvo[h0 : h0 + 2]),
                "gsc": np.ascontiguousarray(
                    np.broadcast_to(gs[h0 : h0 + 2, None], (2, 128))
                ),
                "aug0": aug0,
            }
        )

    res = bass_utils.run_bass_kernel_spmd(nc, in_maps, core_ids=list(range(NCORES)))

    out = np.zeros((B, S, E), np.float32)
    for c in range(NCORES):
        out[c // 4] += res.results[c]["out"].T
    return out
